# revision 38
# baseline (speedup 1.0000x reference)
"""Additive attention kernel for Trainium2 (8 NeuronCores, data-parallel over batch).

reference computation (per batch b):
    h     = tanh(key @ Wk.T + query @ Wq.T + bias)        # (L, H)
    score = h @ w_score (+ b_score)                        # (L,)
    attn  = softmax(score)                                 # (L,)  [b_score cancels]
    ctx   = attn @ key                                     # (H,)

Per-core strategy (4 batches/core, measured ~640 us/core on HW):
  - Projections in transposed layout psum[o, n] = W.T-tile.T @ x.T-tile, in
    bf16 (1 cycle/row on the PE; fp32/f32r streams at 2 cycles/row, which
    makes the projections alone cost ~875us/core). Accumulation is fp32 in
    PSUM, so the error is bf16-input-rounding only (~3e-3 relative overall).
  - x.T tiles are produced by PE transposes (bf16 identity matmuls) with one
    PSUM bank per 128-token strip and a single DVE copy-back per strip. (The
    XBAR DMA transpose is 2x cheaper on paper but corrupts data when
    transpose-DMAs overlap ordinary copy-DMAs — known HW bug — so it is not
    used.)
  - tanh(+bias) on ACT gives h.T tiles [o, n] in bf16; score accumulates as a
    full-width row matmul [1, n] (moving operand = h.T, N=512), software-
    pipelined one o-chunk behind the projections so the PE never waits on
    ACT. Four tiny f32r matmuls then scatter the row into column layout
    [128, 1] per 128 tokens (l on partitions) for softmax and context.
  - Context: ctx_u[1, h'] += exp-col.T @ key[l, h'] in bf16, accumulated in
    fp32 PSUM across the whole batch; the bf16 rounding of exp/key averages
    out over 2048 tokens so it adds ~1e-4, not 2e-3.
  - Softmax skips the max-shift (scores bounded by ||w_score||_1 ~ 26; exp is
    safe in fp32 and the result is identical after normalization). b_score
    drops out of softmax entirely.
"""

import sys
from contextlib import ExitStack

import numpy as np

sys.path.insert(0, "/opt/trn_rl_repo")

import concourse.tile as tile  # noqa: E402
from concourse import bacc, mybir  # noqa: E402
from concourse.bass_utils import run_bass_kernel_spmd  # noqa: E402
from concourse.masks import make_identity  # noqa: E402

B, L, H = 32, 2048, 1024
NCORES = 8
BLOC = B // NCORES  # batches per core
P = 128
HC = H // P  # h (and o) chunks of 128
NT = 512  # token tile
NSUB = NT // P  # 128-token subtiles per tile

F32 = mybir.dt.float32
F32R = mybir.dt.float32r
BF16 = mybir.dt.bfloat16
TANH = mybir.ActivationFunctionType.Tanh
EXP = mybir.ActivationFunctionType.Exp


def build_nc(b_loc=BLOC, l=L):
    tpb = l // NT  # token tiles per batch
    assert l % NT == 0

    nc = bacc.Bacc("TRN2", target_bir_lowering=False, debug=False, num_devices=NCORES)
    q_d = nc.declare_dram_parameter("qbf", [b_loc, l, H], BF16, isOutput=False)
    kbf_d = nc.declare_dram_parameter("kbf", [b_loc, l, H], BF16, isOutput=False)
    wq_d = nc.declare_dram_parameter("wqT", [H, H], BF16, isOutput=False)  # Wq.T
    wk_d = nc.declare_dram_parameter("wkT", [H, H], BF16, isOutput=False)  # Wk.T
    bias_d = nc.declare_dram_parameter("bias_c", [P, HC], F32, isOutput=False)
    wsc_d = nc.declare_dram_parameter("wsc_c", [P, HC], BF16, isOutput=False)
    ctx_d = nc.declare_dram_parameter("ctx", [b_loc, H], F32, isOutput=True)
    attn_d = nc.declare_dram_parameter("attn", [b_loc, l], F32, isOutput=True)

    with tile.TileContext(nc) as tc, ExitStack() as ctx:
        singles = ctx.enter_context(tc.tile_pool(name="singles", bufs=1))
        qnat_p = ctx.enter_context(tc.tile_pool(name="qnat", bufs=3))
        kbf_p = ctx.enter_context(tc.tile_pool(name="kbfp", bufs=3))
        kT_p = ctx.enter_context(tc.tile_pool(name="kTp", bufs=3))
        qT_p = ctx.enter_context(tc.tile_pool(name="qTp", bufs=3))
        hact_p = ctx.enter_context(tc.tile_pool(name="hact", bufs=3))
        small_p = ctx.enter_context(tc.tile_pool(name="small", bufs=2))
        exp_p = ctx.enter_context(tc.tile_pool(name="expp", bufs=2))
        expr_p = ctx.enter_context(tc.tile_pool(name="exprp", bufs=3))
        ppsum = ctx.enter_context(tc.tile_pool(name="ppsum", bufs=3, space="PSUM"))
        tpsum = ctx.enter_context(tc.tile_pool(name="tpsum", bufs=2, space="PSUM"))
        spsum = ctx.enter_context(tc.tile_pool(name="spsum", bufs=1, space="PSUM"))
        cpsum = ctx.enter_context(tc.tile_pool(name="cpsum", bufs=2, space="PSUM"))

        # ---- first-tile loads go ahead of the weights, as independent
        # per-strip tiles so each strip's transposes only wait on their own
        # 0.5 MB DMA (Tile dependency tracking is tile-granular) ----
        firstp = ctx.enter_context(tc.tile_pool(name="firstp", bufs=1))
        k0s, q0s = [], []
        for s in range(NSUB):
            stok = slice(s * P, (s + 1) * P)
            kt0 = firstp.tile([P, H], BF16, name=f"k0s{s}")
            nc.sync.dma_start(kt0[:], kbf_d.ap()[0, stok, :])
            k0s.append(kt0)
            qt0 = firstp.tile([P, H], BF16, name=f"q0s{s}")
            nc.sync.dma_start(qt0[:], q_d.ap()[0, stok, :])
            q0s.append(qt0)

        # ---- constants (small ones first; weights split per o-chunk so the
        # first projection can start before all weights land) ----
        bias_sb = singles.tile([P, HC], F32)
        nc.sync.dma_start(bias_sb[:], bias_d.ap())
        wsc_sb = singles.tile([P, HC], BF16)
        nc.sync.dma_start(wsc_sb[:], wsc_d.ap())
        wq_sb = singles.tile([P, HC, H], BF16)
        wk_sb = singles.tile([P, HC, H], BF16)
        for oc in range(HC):
            osl = slice(oc * P, (oc + 1) * P)
            nc.sync.dma_start(
                wk_sb[:, :, osl], wk_d.ap().rearrange("(hc p) o -> p hc o", p=P)[:, :, osl]
            )
            nc.sync.dma_start(
                wq_sb[:, :, osl], wq_d.ap().rearrange("(hc p) o -> p hc o", p=P)[:, :, osl]
            )
        ident_f = singles.tile([P, P], F32)
        make_identity(nc, ident_f[:])
        ident_b = singles.tile([P, P], BF16)
        nc.vector.tensor_copy(out=ident_b[:], in_=ident_f[:])
        ones_col = singles.tile([P, 1], F32)
        nc.vector.memset(ones_col[:], 1.0)
        ones_row = singles.tile([1, P], F32)
        nc.vector.memset(ones_row[:], 1.0)
        one_one = singles.tile([1, 1], F32)
        nc.vector.memset(one_one[:], 1.0)
        ones_two_f = singles.tile([1, 2], F32)
        nc.vector.memset(ones_two_f[:], 1.0)
        ones_two = singles.tile([1, 2], F32R)
        nc.vector.tensor_copy(out=ones_two[:], in_=ones_two_f[:])

        for b in range(b_loc):
            exp_sb = exp_p.tile([P, tpb * NSUB], F32, tag="exps")
            ctx_ps = [
                cpsum.tile([1, 512], F32, tag="ctx", name=f"ctx_ps{ht}")
                for ht in range(2)
            ]
            for t in range(tpb):
                tok = slice(t * NT, (t + 1) * NT)
                # ---- loads: bf16 q/k (projection + context paths) ----
                if b == 0 and t == 0:
                    def k_strip(s):
                        return k0s[s][:]

                    def q_strip(s):
                        return q0s[s][:]
                else:
                    q_nat = qnat_p.tile([P, NSUB, H], BF16, tag="qn")
                    nc.sync.dma_start(
                        q_nat[:], q_d.ap()[b, tok, :].rearrange("(s p) h -> p s h", p=P)
                    )
                    kbf_nat = kbf_p.tile([P, NSUB, H], BF16, tag="kbn")
                    nc.sync.dma_start(
                        kbf_nat[:],
                        kbf_d.ap()[b, tok, :].rearrange("(s p) h -> p s h", p=P),
                    )

                    def k_strip(s, _k=kbf_nat):
                        return _k[:, s, :]

                    def q_strip(s, _q=q_nat):
                        return _q[:, s, :]

                # ---- PE transposes (bf16, 1 cyc/row): x[128n,128h] -> [128h,128n]
                # all 8 h-chunks of one 128-token strip share one PSUM bank
                # (8 x 128 bf16 cols = 2 KB) -> one DVE copy-back per strip ----
                kT = kT_p.tile([P, HC, NT], BF16, tag="kT")
                qT = qT_p.tile([P, HC, NT], BF16, tag="qT")
                for get, dst in ((k_strip, kT), (q_strip, qT)):
                    for s in range(NSUB):
                        tp = tpsum.tile([P, HC * P], BF16, tag="tp")
                        for hc in range(HC):
                            nc.tensor.transpose(
                                tp[:, hc * P : (hc + 1) * P],
                                get(s)[:, hc * P : (hc + 1) * P],
                                ident_b[:],
                            )
                        nc.vector.tensor_copy(
                            out=dst[:, :, s * P : (s + 1) * P],
                            in_=tp[:].rearrange("p (j n) -> p j n", j=HC),
                        )

                # ---- projections + tanh + score-row ----
                # the score-row matmul for chunk c is emitted after chunk c+1's
                # projections, so the PE never stalls waiting for tanh on ACT
                srow = spsum.tile([1, NT], F32, tag="srow")
                ha_prev = None
                for oc in range(HC):
                    pp = ppsum.tile([P, NT], F32, tag="pp")
                    for hc in range(HC):
                        nc.tensor.matmul(
                            pp[:],
                            wk_sb[:, hc, oc * P : (oc + 1) * P],
                            kT[:, hc, :],
                            start=(hc == 0),
                            stop=False,
                        )
                    for hc in range(HC):
                        nc.tensor.matmul(
                            pp[:],
                            wq_sb[:, hc, oc * P : (oc + 1) * P],
                            qT[:, hc, :],
                            start=False,
                            stop=(hc == HC - 1),
                        )
                    if ha_prev is not None:
                        nc.tensor.matmul(
                            srow[:],
                            wsc_sb[:, oc - 1 : oc],
                            ha_prev[:],
                            start=(oc == 1),
                            stop=False,
                        )
                    ha = hact_p.tile([P, NT], BF16, tag="ha")
                    nc.scalar.activation(
                        ha[:], pp[:], TANH, bias=bias_sb[:, oc : oc + 1], scale=1.0
                    )
                    ha_prev = ha
                nc.tensor.matmul(
                    srow[:],
                    wsc_sb[:, HC - 1 : HC],
                    ha_prev[:],
                    start=False,
                    stop=True,
                )

                # ---- scatter score row [1, 512] into columns [128, 2*NSUB]
                # (f32r needs an even dst free count, so each scatter matmul
                # writes the column twice via a [1, 2] ones moving operand) ----
                srow_sb = small_p.tile([1, NT], F32R, tag="srowsb")
                nc.vector.tensor_copy(out=srow_sb[:], in_=srow[:])
                sc = spsum.tile([P, 2 * NSUB], F32, tag="srow", name="sc")
                for j in range(NSUB):
                    nc.tensor.matmul(
                        sc[:, 2 * j : 2 * j + 2],
                        srow_sb[0:1, j * P : (j + 1) * P],
                        ones_two[:],
                        start=True,
                        stop=True,
                    )

                # ---- exp (no max-shift needed; scores bounded) ----
                sc_cols = sc[:].rearrange("p (j two) -> p j two", two=2)[:, :, 0]
                nc.scalar.activation(
                    exp_sb[:, t * NSUB : (t + 1) * NSUB], sc_cols, EXP
                )
                exp_r = expr_p.tile([P, NSUB], BF16, tag="expr")
                nc.scalar.activation(exp_r[:], sc_cols, EXP)

                # ---- context accumulation: ctx_u[1, h'] += exp_l @ key[l, h'] ----
                for ht in range(2):
                    for s in range(NSUB):
                        nc.tensor.matmul(
                            ctx_ps[ht][:],
                            exp_r[:, s : s + 1],
                            k_strip(s)[:, ht * 512 : (ht + 1) * 512],
                            start=(t == 0 and s == 0),
                            stop=(t == tpb - 1 and s == NSUB - 1),
                        )

            # ---- batch epilogue: Z, normalize, write out ----
            zpart = small_p.tile([P, 1], F32, tag="zpart")
            nc.vector.reduce_sum(zpart[:], exp_sb[:], axis=mybir.AxisListType.X)
            z_ps = spsum.tile([1, 1], F32, tag="srow", name="z_ps")
            nc.tensor.matmul(z_ps[:], zpart[:], ones_col[:], start=True, stop=True)
            zinv = small_p.tile([1, 1], F32, tag="zinv")
            nc.vector.reciprocal(zinv[:], z_ps[:])

            ctx_sb = small_p.tile([1, H], F32, tag="ctxsb")
            for ht in range(2):
                nc.vector.tensor_scalar_mul(
                    ctx_sb[:, ht * 512 : (ht + 1) * 512], ctx_ps[ht][:], zinv[:]
                )
            nc.sync.dma_start(ctx_d.ap()[b : b + 1, :], ctx_sb[:])

            # broadcast 1/Z to all 128 partitions via ones-column matmul
            zb_ps = spsum.tile([P, 1], F32, tag="srow", name="zb_ps")
            nc.tensor.matmul(zb_ps[:], ones_row[:], zinv[:], start=True, stop=True)
            zb_sb = small_p.tile([P, 1], F32, tag="zb")
            nc.vector.tensor_copy(out=zb_sb[:], in_=zb_ps[:])
            attn_sb = small_p.tile([P, tpb * NSUB], F32, tag="attnsb")
            nc.vector.tensor_scalar_mul(attn_sb[:], exp_sb[:], zb_sb[:])

            # transpose [128, tpb*NSUB] -> [tpb*NSUB, 128] for contiguous DMA out
            at_ps = spsum.tile([tpb * NSUB, P], F32, tag="srow", name="at_ps")
            nc.tensor.transpose(at_ps[:], attn_sb[:], ident_f[:])
            at_sb = small_p.tile([tpb * NSUB, P], F32, tag="atsb")
            nc.vector.tensor_copy(out=at_sb[:], in_=at_ps[:])
            nc.sync.dma_start(
                attn_d.ap()[b, :].rearrange("(c p) -> c p", p=P), at_sb[:]
            )

    nc.compile()
    return nc


_NC_CACHE = {}


def _get_nc(b_loc=BLOC, l=L):
    key = (b_loc, l)
    if key not in _NC_CACHE:
        _NC_CACHE[key] = build_nc(b_loc, l)
    return _NC_CACHE[key]


def _prep_host(query, key, Wq, Wk, bias, w_score):
    import ml_dtypes

    bf16 = ml_dtypes.bfloat16
    query = np.asarray(query, dtype=np.float32)
    key = np.ascontiguousarray(np.asarray(key, dtype=np.float32))
    q_bf = np.ascontiguousarray(query.astype(bf16))
    k_bf = np.ascontiguousarray(key.astype(bf16))
    del key
    Wq = np.asarray(Wq, dtype=np.float32)
    Wk = np.asarray(Wk, dtype=np.float32)
    bias = np.asarray(bias, dtype=np.float32)
    w_score = np.asarray(w_score, dtype=np.float32)
    wqT = np.ascontiguousarray(Wq.T.astype(bf16))
    wkT = np.ascontiguousarray(Wk.T.astype(bf16))
    bias_c = np.ascontiguousarray(bias.reshape(HC, P).T)
    wsc_c = np.ascontiguousarray(w_score.reshape(HC, P).T.astype(bf16))
    return q_bf, k_bf, wqT, wkT, bias_c, wsc_c


def _in_maps(q_bf, k_bf, wqT, wkT, bias_c, wsc_c):
    maps = []
    for c in range(NCORES):
        sl = slice(c * BLOC, (c + 1) * BLOC)
        maps.append(
            {
                "qbf": q_bf[sl],
                "kbf": k_bf[sl],
                "wqT": wqT,
                "wkT": wkT,
                "bias_c": bias_c,
                "wsc_c": wsc_c,
            }
        )
    return maps


def _gather(results):
    ctx = np.concatenate([r["ctx"] for r in results], axis=0)[:, None, :]
    attn = np.concatenate([r["attn"] for r in results], axis=0)
    return ctx.astype(np.float32), attn.astype(np.float32)


def kernel(query, key, Wq, Wk, bias, w_score, b_score=None):
    prep = _prep_host(query, key, Wq, Wk, bias, w_score)
    nc = _get_nc()
    res = run_bass_kernel_spmd(nc, _in_maps(*prep), core_ids=list(range(NCORES)))
    return _gather(res.results)


def _ensure_ntff_hook():
    """Synthesize antenv.axon_hooks with a ctypes NTFF hook if the image lacks it."""
    try:
        from antenv import axon_hooks  # noqa: F401

        return
    except ImportError:
        pass
    import contextlib
    import ctypes
    import types

    import antenv

    holder = {"hook": None}
    mod = types.ModuleType("antenv.axon_hooks")
    mod.set_axon_ntff_profile_hook = lambda h: holder.__setitem__("hook", h)
    mod.get_axon_ntff_profile_hook = lambda: holder["hook"]
    sys.modules["antenv.axon_hooks"] = mod
    antenv.axon_hooks = mod

    so_path = "/opt/axon/libaxon_pjrt.so"
    try:
        lib = ctypes.CDLL(so_path)
        if not hasattr(lib, "axon_start_nrt_profile"):
            return
        lib.axon_start_nrt_profile.argtypes = [
            ctypes.POINTER(ctypes.c_int64),
            ctypes.c_size_t,
        ]
        lib.axon_start_nrt_profile.restype = ctypes.c_int64
        lib.axon_stop_nrt_profile.argtypes = [ctypes.c_char_p]
        lib.axon_stop_nrt_profile.restype = ctypes.c_int64
    except OSError:
        return

    @contextlib.contextmanager
    def _hook(output_dir, device_ids):
        import jax

        jax.devices()
        if device_ids:
            ids = (ctypes.c_int64 * len(device_ids))(*device_ids)
            rc = lib.axon_start_nrt_profile(ids, len(device_ids))
        else:
            rc = lib.axon_start_nrt_profile(None, 0)
        if rc != 0:
            raise RuntimeError(f"axon_start_nrt_profile rc={rc}")
        try:
            yield
        finally:
            n = lib.axon_stop_nrt_profile(str(output_dir).encode())
            print(f"profile: {n} file(s) written to {output_dir}", file=sys.stderr)

    holder["hook"] = _hook


def kernel_profiled(query, key, Wq, Wk, bias, w_score, b_score=None, **trace_kwargs):
    """Like kernel() but with NTFF tracing; returns (outputs, BassKernelResults)."""
    _ensure_ntff_hook()
    from concourse import bass_utils as _bu

    _bu.upload_artifacts = lambda tmpdir: f"(local){tmpdir}"  # no bucket in this env
    prep = _prep_host(query, key, Wq, Wk, bias, w_score)
    nc = _get_nc()
    res = run_bass_kernel_spmd(
        nc, _in_maps(*prep), core_ids=list(range(NCORES)), trace=True, **trace_kwargs
    )
    return _gather(res.results), res


# revision 39
# speedup vs baseline: 1.0188x; 1.0188x over previous
"""Additive attention kernel for Trainium2 (8 NeuronCores, data-parallel over batch).

reference computation (per batch b):
    h     = tanh(key @ Wk.T + query @ Wq.T + bias)        # (L, H)
    score = h @ w_score (+ b_score)                        # (L,)
    attn  = softmax(score)                                 # (L,)  [b_score cancels]
    ctx   = attn @ key                                     # (H,)

Per-core strategy (4 batches/core, measured ~640 us/core on HW):
  - Projections in transposed layout psum[o, n] = W.T-tile.T @ x.T-tile, in
    bf16 (1 cycle/row on the PE; fp32/f32r streams at 2 cycles/row, which
    makes the projections alone cost ~875us/core). Accumulation is fp32 in
    PSUM, so the error is bf16-input-rounding only (~3e-3 relative overall).
  - x.T tiles are produced by PE transposes (bf16 identity matmuls) with one
    PSUM bank per 128-token strip and a single DVE copy-back per strip. (The
    XBAR DMA transpose is 2x cheaper on paper but corrupts data when
    transpose-DMAs overlap ordinary copy-DMAs — known HW bug — so it is not
    used.)
  - tanh(+bias) on ACT gives h.T tiles [o, n] in bf16; score accumulates as a
    full-width row matmul [1, n] (moving operand = h.T, N=512), software-
    pipelined one o-chunk behind the projections so the PE never waits on
    ACT. Four tiny f32r matmuls then scatter the row into column layout
    [128, 1] per 128 tokens (l on partitions) for softmax and context.
  - Context: ctx_u[1, h'] += exp-col.T @ key[l, h'] in bf16, accumulated in
    fp32 PSUM across the whole batch; the bf16 rounding of exp/key averages
    out over 2048 tokens so it adds ~1e-4, not 2e-3.
  - Softmax skips the max-shift (scores bounded by ||w_score||_1 ~ 26; exp is
    safe in fp32 and the result is identical after normalization). b_score
    drops out of softmax entirely.
"""

import sys
from contextlib import ExitStack

import numpy as np

sys.path.insert(0, "/opt/trn_rl_repo")

import concourse.tile as tile  # noqa: E402
from concourse import bacc, mybir  # noqa: E402
from concourse.bass_utils import run_bass_kernel_spmd  # noqa: E402
from concourse.masks import make_identity  # noqa: E402

B, L, H = 32, 2048, 1024
NCORES = 8
BLOC = B // NCORES  # batches per core
P = 128
HC = H // P  # h (and o) chunks of 128
NT = 512  # token tile
NSUB = NT // P  # 128-token subtiles per tile

F32 = mybir.dt.float32
F32R = mybir.dt.float32r
BF16 = mybir.dt.bfloat16
TANH = mybir.ActivationFunctionType.Tanh
EXP = mybir.ActivationFunctionType.Exp


def build_nc(b_loc=BLOC, l=L):
    tpb = l // NT  # token tiles per batch
    assert l % NT == 0

    nc = bacc.Bacc("TRN2", target_bir_lowering=False, debug=False, num_devices=NCORES)
    q_d = nc.declare_dram_parameter("qbf", [b_loc, l, H], BF16, isOutput=False)
    kbf_d = nc.declare_dram_parameter("kbf", [b_loc, l, H], BF16, isOutput=False)
    wq_d = nc.declare_dram_parameter("wqT", [H, H], BF16, isOutput=False)  # Wq.T
    wk_d = nc.declare_dram_parameter("wkT", [H, H], BF16, isOutput=False)  # Wk.T
    bias_d = nc.declare_dram_parameter("bias_c", [P, HC], F32, isOutput=False)
    wsc_d = nc.declare_dram_parameter("wsc_c", [P, HC], BF16, isOutput=False)
    ctx_d = nc.declare_dram_parameter("ctx", [b_loc, H], F32, isOutput=True)
    attn_d = nc.declare_dram_parameter("attn", [b_loc, l], F32, isOutput=True)

    with tile.TileContext(nc) as tc, ExitStack() as ctx:
        singles = ctx.enter_context(tc.tile_pool(name="singles", bufs=1))
        qnat_p = ctx.enter_context(tc.tile_pool(name="qnat", bufs=3))
        kbf_p = ctx.enter_context(tc.tile_pool(name="kbfp", bufs=3))
        kT_p = ctx.enter_context(tc.tile_pool(name="kTp", bufs=3))
        qT_p = ctx.enter_context(tc.tile_pool(name="qTp", bufs=3))
        hact_p = ctx.enter_context(tc.tile_pool(name="hact", bufs=3))
        small_p = ctx.enter_context(tc.tile_pool(name="small", bufs=2))
        exp_p = ctx.enter_context(tc.tile_pool(name="expp", bufs=2))
        expr_p = ctx.enter_context(tc.tile_pool(name="exprp", bufs=3))
        ppsum = ctx.enter_context(tc.tile_pool(name="ppsum", bufs=2, space="PSUM"))
        tpsum = ctx.enter_context(tc.tile_pool(name="tpsum", bufs=3, space="PSUM"))
        spsum = ctx.enter_context(tc.tile_pool(name="spsum", bufs=1, space="PSUM"))
        cpsum = ctx.enter_context(tc.tile_pool(name="cpsum", bufs=2, space="PSUM"))

        # ---- first-tile loads go ahead of the weights, as independent
        # per-strip tiles so each strip's transposes only wait on their own
        # 0.5 MB DMA (Tile dependency tracking is tile-granular) ----
        firstp = ctx.enter_context(tc.tile_pool(name="firstp", bufs=1))
        k0s, q0s = [], []
        for s in range(NSUB):
            stok = slice(s * P, (s + 1) * P)
            kt0 = firstp.tile([P, H], BF16, name=f"k0s{s}")
            nc.sync.dma_start(kt0[:], kbf_d.ap()[0, stok, :])
            k0s.append(kt0)
            qt0 = firstp.tile([P, H], BF16, name=f"q0s{s}")
            nc.sync.dma_start(qt0[:], q_d.ap()[0, stok, :])
            q0s.append(qt0)

        # ---- constants (small ones first; weights split per o-chunk so the
        # first projection can start before all weights land) ----
        bias_sb = singles.tile([P, HC], F32)
        nc.sync.dma_start(bias_sb[:], bias_d.ap())
        wsc_sb = singles.tile([P, HC], BF16)
        nc.sync.dma_start(wsc_sb[:], wsc_d.ap())
        wq_sb = singles.tile([P, HC, H], BF16)
        wk_sb = singles.tile([P, HC, H], BF16)
        for oc in range(HC):
            osl = slice(oc * P, (oc + 1) * P)
            nc.sync.dma_start(
                wk_sb[:, :, osl], wk_d.ap().rearrange("(hc p) o -> p hc o", p=P)[:, :, osl]
            )
            nc.sync.dma_start(
                wq_sb[:, :, osl], wq_d.ap().rearrange("(hc p) o -> p hc o", p=P)[:, :, osl]
            )
        ident_f = singles.tile([P, P], F32)
        make_identity(nc, ident_f[:])
        ident_b = singles.tile([P, P], BF16)
        nc.vector.tensor_copy(out=ident_b[:], in_=ident_f[:])
        ones_col = singles.tile([P, 1], F32)
        nc.vector.memset(ones_col[:], 1.0)
        ones_row = singles.tile([1, P], F32)
        nc.vector.memset(ones_row[:], 1.0)
        one_one = singles.tile([1, 1], F32)
        nc.vector.memset(one_one[:], 1.0)
        ones_two_f = singles.tile([1, 2], F32)
        nc.vector.memset(ones_two_f[:], 1.0)
        ones_two = singles.tile([1, 2], F32R)
        nc.vector.tensor_copy(out=ones_two[:], in_=ones_two_f[:])

        for b in range(b_loc):
            exp_sb = exp_p.tile([P, tpb * NSUB], F32, tag="exps")
            ctx_ps = [
                cpsum.tile([1, 512], F32, tag="ctx", name=f"ctx_ps{ht}")
                for ht in range(2)
            ]
            for t in range(tpb):
                tok = slice(t * NT, (t + 1) * NT)
                # ---- loads: bf16 q/k (projection + context paths) ----
                if b == 0 and t == 0:
                    def k_strip(s):
                        return k0s[s][:]

                    def q_strip(s):
                        return q0s[s][:]
                else:
                    q_nat = qnat_p.tile([P, NSUB, H], BF16, tag="qn")
                    nc.sync.dma_start(
                        q_nat[:], q_d.ap()[b, tok, :].rearrange("(s p) h -> p s h", p=P)
                    )
                    kbf_nat = kbf_p.tile([P, NSUB, H], BF16, tag="kbn")
                    nc.sync.dma_start(
                        kbf_nat[:],
                        kbf_d.ap()[b, tok, :].rearrange("(s p) h -> p s h", p=P),
                    )

                    def k_strip(s, _k=kbf_nat):
                        return _k[:, s, :]

                    def q_strip(s, _q=q_nat):
                        return _q[:, s, :]

                # ---- PE transposes (bf16, 1 cyc/row): x[128n,128h] -> [128h,128n]
                # all 8 h-chunks of one 128-token strip share one PSUM bank
                # (8 x 128 bf16 cols = 2 KB) -> one DVE copy-back per strip ----
                kT = kT_p.tile([P, HC, NT], BF16, tag="kT")
                qT = qT_p.tile([P, HC, NT], BF16, tag="qT")
                for get, dst in ((k_strip, kT), (q_strip, qT)):
                    for s in range(NSUB):
                        tp = tpsum.tile([P, HC * P], BF16, tag="tp")
                        for hc in range(HC):
                            nc.tensor.transpose(
                                tp[:, hc * P : (hc + 1) * P],
                                get(s)[:, hc * P : (hc + 1) * P],
                                ident_b[:],
                            )
                        nc.vector.tensor_copy(
                            out=dst[:, :, s * P : (s + 1) * P],
                            in_=tp[:].rearrange("p (j n) -> p j n", j=HC),
                        )

                # ---- projections + tanh + score-row ----
                # the score-row matmul for chunk c is emitted after chunk c+1's
                # projections, so the PE never stalls waiting for tanh on ACT
                srow = spsum.tile([1, NT], F32, tag="srow")
                ha_prev = None
                for oc in range(HC):
                    pp = ppsum.tile([P, NT], F32, tag="pp")
                    for hc in range(HC):
                        nc.tensor.matmul(
                            pp[:],
                            wk_sb[:, hc, oc * P : (oc + 1) * P],
                            kT[:, hc, :],
                            start=(hc == 0),
                            stop=False,
                        )
                    for hc in range(HC):
                        nc.tensor.matmul(
                            pp[:],
                            wq_sb[:, hc, oc * P : (oc + 1) * P],
                            qT[:, hc, :],
                            start=False,
                            stop=(hc == HC - 1),
                        )
                    if ha_prev is not None:
                        nc.tensor.matmul(
                            srow[:],
                            wsc_sb[:, oc - 1 : oc],
                            ha_prev[:],
                            start=(oc == 1),
                            stop=False,
                        )
                    ha = hact_p.tile([P, NT], BF16, tag="ha")
                    nc.scalar.activation(
                        ha[:], pp[:], TANH, bias=bias_sb[:, oc : oc + 1], scale=1.0
                    )
                    ha_prev = ha
                nc.tensor.matmul(
                    srow[:],
                    wsc_sb[:, HC - 1 : HC],
                    ha_prev[:],
                    start=False,
                    stop=True,
                )

                # ---- scatter score row [1, 512] into columns [128, 2*NSUB]
                # (f32r needs an even dst free count, so each scatter matmul
                # writes the column twice via a [1, 2] ones moving operand) ----
                srow_sb = small_p.tile([1, NT], F32R, tag="srowsb")
                nc.vector.tensor_copy(out=srow_sb[:], in_=srow[:])
                sc = spsum.tile([P, 2 * NSUB], F32, tag="srow", name="sc")
                for j in range(NSUB):
                    nc.tensor.matmul(
                        sc[:, 2 * j : 2 * j + 2],
                        srow_sb[0:1, j * P : (j + 1) * P],
                        ones_two[:],
                        start=True,
                        stop=True,
                    )

                # ---- exp (no max-shift needed; scores bounded) ----
                sc_cols = sc[:].rearrange("p (j two) -> p j two", two=2)[:, :, 0]
                nc.scalar.activation(
                    exp_sb[:, t * NSUB : (t + 1) * NSUB], sc_cols, EXP
                )
                exp_r = expr_p.tile([P, NSUB], BF16, tag="expr")
                nc.scalar.activation(exp_r[:], sc_cols, EXP)

                # ---- context accumulation: ctx_u[1, h'] += exp_l @ key[l, h'] ----
                for ht in range(2):
                    for s in range(NSUB):
                        nc.tensor.matmul(
                            ctx_ps[ht][:],
                            exp_r[:, s : s + 1],
                            k_strip(s)[:, ht * 512 : (ht + 1) * 512],
                            start=(t == 0 and s == 0),
                            stop=(t == tpb - 1 and s == NSUB - 1),
                        )

            # ---- batch epilogue: Z, normalize, write out ----
            zpart = small_p.tile([P, 1], F32, tag="zpart")
            nc.vector.reduce_sum(zpart[:], exp_sb[:], axis=mybir.AxisListType.X)
            z_ps = spsum.tile([1, 1], F32, tag="srow", name="z_ps")
            nc.tensor.matmul(z_ps[:], zpart[:], ones_col[:], start=True, stop=True)
            zinv = small_p.tile([1, 1], F32, tag="zinv")
            nc.vector.reciprocal(zinv[:], z_ps[:])

            ctx_sb = small_p.tile([1, H], F32, tag="ctxsb")
            for ht in range(2):
                nc.vector.tensor_scalar_mul(
                    ctx_sb[:, ht * 512 : (ht + 1) * 512], ctx_ps[ht][:], zinv[:]
                )
            nc.sync.dma_start(ctx_d.ap()[b : b + 1, :], ctx_sb[:])

            # broadcast 1/Z to all 128 partitions via ones-column matmul
            zb_ps = spsum.tile([P, 1], F32, tag="srow", name="zb_ps")
            nc.tensor.matmul(zb_ps[:], ones_row[:], zinv[:], start=True, stop=True)
            zb_sb = small_p.tile([P, 1], F32, tag="zb")
            nc.vector.tensor_copy(out=zb_sb[:], in_=zb_ps[:])
            attn_sb = small_p.tile([P, tpb * NSUB], F32, tag="attnsb")
            nc.vector.tensor_scalar_mul(attn_sb[:], exp_sb[:], zb_sb[:])

            # transpose [128, tpb*NSUB] -> [tpb*NSUB, 128] for contiguous DMA out
            at_ps = spsum.tile([tpb * NSUB, P], F32, tag="srow", name="at_ps")
            nc.tensor.transpose(at_ps[:], attn_sb[:], ident_f[:])
            at_sb = small_p.tile([tpb * NSUB, P], F32, tag="atsb")
            nc.vector.tensor_copy(out=at_sb[:], in_=at_ps[:])
            nc.sync.dma_start(
                attn_d.ap()[b, :].rearrange("(c p) -> c p", p=P), at_sb[:]
            )

    nc.compile()
    return nc


_NC_CACHE = {}


def _get_nc(b_loc=BLOC, l=L):
    key = (b_loc, l)
    if key not in _NC_CACHE:
        _NC_CACHE[key] = build_nc(b_loc, l)
    return _NC_CACHE[key]


def _prep_host(query, key, Wq, Wk, bias, w_score):
    import ml_dtypes

    bf16 = ml_dtypes.bfloat16
    query = np.asarray(query, dtype=np.float32)
    key = np.ascontiguousarray(np.asarray(key, dtype=np.float32))
    q_bf = np.ascontiguousarray(query.astype(bf16))
    k_bf = np.ascontiguousarray(key.astype(bf16))
    del key
    Wq = np.asarray(Wq, dtype=np.float32)
    Wk = np.asarray(Wk, dtype=np.float32)
    bias = np.asarray(bias, dtype=np.float32)
    w_score = np.asarray(w_score, dtype=np.float32)
    wqT = np.ascontiguousarray(Wq.T.astype(bf16))
    wkT = np.ascontiguousarray(Wk.T.astype(bf16))
    bias_c = np.ascontiguousarray(bias.reshape(HC, P).T)
    wsc_c = np.ascontiguousarray(w_score.reshape(HC, P).T.astype(bf16))
    return q_bf, k_bf, wqT, wkT, bias_c, wsc_c


def _in_maps(q_bf, k_bf, wqT, wkT, bias_c, wsc_c):
    maps = []
    for c in range(NCORES):
        sl = slice(c * BLOC, (c + 1) * BLOC)
        maps.append(
            {
                "qbf": q_bf[sl],
                "kbf": k_bf[sl],
                "wqT": wqT,
                "wkT": wkT,
                "bias_c": bias_c,
                "wsc_c": wsc_c,
            }
        )
    return maps


def _gather(results):
    ctx = np.concatenate([r["ctx"] for r in results], axis=0)[:, None, :]
    attn = np.concatenate([r["attn"] for r in results], axis=0)
    return ctx.astype(np.float32), attn.astype(np.float32)


def kernel(query, key, Wq, Wk, bias, w_score, b_score=None):
    prep = _prep_host(query, key, Wq, Wk, bias, w_score)
    nc = _get_nc()
    res = run_bass_kernel_spmd(nc, _in_maps(*prep), core_ids=list(range(NCORES)))
    return _gather(res.results)


def _ensure_ntff_hook():
    """Synthesize antenv.axon_hooks with a ctypes NTFF hook if the image lacks it."""
    try:
        from antenv import axon_hooks  # noqa: F401

        return
    except ImportError:
        pass
    import contextlib
    import ctypes
    import types

    import antenv

    holder = {"hook": None}
    mod = types.ModuleType("antenv.axon_hooks")
    mod.set_axon_ntff_profile_hook = lambda h: holder.__setitem__("hook", h)
    mod.get_axon_ntff_profile_hook = lambda: holder["hook"]
    sys.modules["antenv.axon_hooks"] = mod
    antenv.axon_hooks = mod

    so_path = "/opt/axon/libaxon_pjrt.so"
    try:
        lib = ctypes.CDLL(so_path)
        if not hasattr(lib, "axon_start_nrt_profile"):
            return
        lib.axon_start_nrt_profile.argtypes = [
            ctypes.POINTER(ctypes.c_int64),
            ctypes.c_size_t,
        ]
        lib.axon_start_nrt_profile.restype = ctypes.c_int64
        lib.axon_stop_nrt_profile.argtypes = [ctypes.c_char_p]
        lib.axon_stop_nrt_profile.restype = ctypes.c_int64
    except OSError:
        return

    @contextlib.contextmanager
    def _hook(output_dir, device_ids):
        import jax

        jax.devices()
        if device_ids:
            ids = (ctypes.c_int64 * len(device_ids))(*device_ids)
            rc = lib.axon_start_nrt_profile(ids, len(device_ids))
        else:
            rc = lib.axon_start_nrt_profile(None, 0)
        if rc != 0:
            raise RuntimeError(f"axon_start_nrt_profile rc={rc}")
        try:
            yield
        finally:
            n = lib.axon_stop_nrt_profile(str(output_dir).encode())
            print(f"profile: {n} file(s) written to {output_dir}", file=sys.stderr)

    holder["hook"] = _hook


def kernel_profiled(query, key, Wq, Wk, bias, w_score, b_score=None, **trace_kwargs):
    """Like kernel() but with NTFF tracing; returns (outputs, BassKernelResults)."""
    _ensure_ntff_hook()
    from concourse import bass_utils as _bu

    _bu.upload_artifacts = lambda tmpdir: f"(local){tmpdir}"  # no bucket in this env
    prep = _prep_host(query, key, Wq, Wk, bias, w_score)
    nc = _get_nc()
    res = run_bass_kernel_spmd(
        nc, _in_maps(*prep), core_ids=list(range(NCORES)), trace=True, **trace_kwargs
    )
    return _gather(res.results), res
